# revision 47
# baseline (speedup 1.0000x reference)
"""3-layer GCN (GCNConv x3) on 8 Trainium2 NeuronCores.

Strategy (node-partitioned, PE scatter-add), V4:
  - Nodes are block-partitioned across the 8 cores by destination id
    (core c owns rows [c*OWN, (c+1)*OWN)).
  - Per layer: each core transforms its own node rows (H @ W, scaled by
    dis = deg^-1/2) into a packed bf16 table shard; ONE 8-wide
    AllGather per layer lands the full node table in each core's DRAM
    (node order == core order, and global pair ids < 25024 fit int16,
    so no segmenting is needed); then each core aggregates messages
    for its own destinations:
      * the table is gathered PAIR-wise: dma_gather needs >=256B
        descriptors, and a bf16 row is only 128B, so each descriptor
        fetches the 256B pair (h[2i], h[2i+1]) and the aggregation
        selects the correct half per edge,
      * scaled+merged into per-destination-block PSUM accumulators via
        selection-matrix matmuls on the tensor engine (even-source
        edges contract against msg cols 0:64, odd-source edges against
        cols 64:128); sel matrices are built by a batched DVE is_equal
        in [edge, dst, chunk] layout against a materialized index
        constant so every operand keeps a real stride-1 last dim and
        the DVE runs in its 2x fast mode,
      * each block's PSUM accumulation chain is one contiguous run in
        PE program order (interleaved open chains, or reopening a
        closed chain with start=False, corrupt results on this HW; the
        single-segment plan keeps every chain contiguous for free).
  - Tail pipelining: as each PSUM bank's chain closes, its epilogue
    (+ t_own, * dis[dst], + bias, relu -> bf16), the NEXT layer's
    transform of those blocks (xbar DMA pair-transposes + bf16 matmuls
    reusing the just-drained PSUM bank), and the bank's ag_in write
    are emitted inline, overlapping the gather-bound remainder of the
    aggregation; only the per-layer AllGather stays exposed.
  - norm = dis[src]*dis[dst] is folded as: table rows pre-scaled by
    dis[src], aggregated output post-scaled by dis[dst]; no per-edge
    scaling needed.

The host side only does graph partitioning / index packing (sorting,
bucketing, degree counts); all tensor math runs on the NeuronCores.
"""

import sys

sys.path.insert(0, "/opt/trn_rl_repo")

import numpy as np
import ml_dtypes

N_NODES = 50000
N_CORES = 8
OWN = N_NODES // N_CORES  # 6250
P = 128
NBLK = (OWN + P - 1) // P  # 49
LAST_ROWS = OWN - (NBLK - 1) * P  # 106
F_IN = 128
F_TAB = 64  # table width, all layers (layer-3 W padded 32->64)
F_OUT = 32
SEG_BOUND = 25000  # int16-safe source bucketing (pair ids < 12500)
G_SEL = 16  # chunks per DVE sel-build op
G_CALL = 48  # chunks per dma_gather call
N_BANK = (NBLK + 7) // 8  # PSUM banks for aggregation (7)
SINGLE_PACKET = False  # single-packet mode hangs SDMA on >1k-descriptor gathers
SKIP_AG = False  # ablation: drop the AllGather collectives
N_QUEUES = 4  # SWDGE queues; gather calls round-robin across them
DMA_SCRATCH = 16384  # dynamic-DMA scratch bytes (ring = /16 descriptors)
MSG_BUFS = 3  # in-flight gather destination tiles
SIM_1CORE = False  # build single-core (no collective) variant for TimelineSim
GATHER_OFF = False  # ablation: skip dma_gather calls (timing only, wrong result)
L_MAX = 3  # ablation: number of GCN layers to build (3 = full model)

BF16 = ml_dtypes.bfloat16


def _wrap_idx(flat_idx):
    """int16 gather-index layout: idx j at [j%16, j//16], replicated x8."""
    w = flat_idx.astype(np.int16).reshape(-1, 16).T  # [16, T*8]
    return np.ascontiguousarray(np.tile(w, (8, 1)))  # [128, T*8]


def _greedy_assign(deg_lo, deg_hi):
    """Greedy vector bin-packing of nodes into (core, block) bins by
    per-segment incoming-edge load. Returns perm (old id -> new id)."""
    tot = deg_lo + deg_hi
    order = np.argsort(-tot, kind="stable")
    nbins = N_CORES * NBLK
    cap = np.zeros(nbins, dtype=np.int64)
    load = np.zeros((nbins, 2), dtype=np.float64)
    capacity = np.full(nbins, P, dtype=np.int64)
    capacity[NBLK - 1 :: NBLK] = LAST_ROWS
    perm = np.empty(N_NODES, dtype=np.int64)
    score = load.max(axis=1) * 4096 + load.sum(axis=1)
    for n in order:
        b = int(np.argmin(score))
        c, blk = b // NBLK, b % NBLK
        perm[n] = c * OWN + blk * P + cap[b]
        cap[b] += 1
        load[b, 0] += deg_lo[n]
        load[b, 1] += deg_hi[n]
        if cap[b] >= capacity[b]:
            score[b] = np.inf
        else:
            score[b] = load[b].max() * 4096 + load[b].sum()
    return perm


def _refine_swaps(perm, src, dst):
    """Swap nodes between cores within the same block (and same core-side,
    so source-segment membership is unchanged) to pull every core's
    per-(block, seg) load under the block's ceil target."""
    deg = np.zeros((N_NODES, 2), dtype=np.int64)
    np.add.at(deg, (dst, 0), 1)
    node_of = np.argsort(perm)  # new id -> old node
    moved = 0
    for cores in (range(N_CORES),):
        for b in range(NBLK):
            rows = LAST_ROWS if b == NBLK - 1 else P
            bins = {
                c: list(node_of[c * OWN + b * P : c * OWN + b * P + rows])
                for c in cores
            }
            load = {
                c: np.array([deg[bins[c], 0].sum(), deg[bins[c], 1].sum()])
                for c in cores
            }
            for g in (0, 1):
                tgt = 128 * int(
                    np.ceil(np.mean([load[c][g] for c in cores]) / P)
                )
                for c in cores:
                    guard = 0
                    while load[c][g] > tgt and guard < 200:
                        guard += 1
                        c2 = min(cores, key=lambda k: load[k][g])
                        if load[c2][g] >= tgt:
                            break
                        need = load[c][g] - tgt
                        room = tgt - load[c2][g]
                        # donor: largest deg_g node; receiver: smallest
                        i = max(range(len(bins[c])), key=lambda i: deg[bins[c][i], g])
                        j = min(range(len(bins[c2])), key=lambda j: deg[bins[c2][j], g])
                        d = deg[bins[c][i], g] - deg[bins[c2][j], g]
                        if d <= 0 or d > need + room:
                            break
                        bins[c][i], bins[c2][j] = bins[c2][j], bins[c][i]
                        dv = deg[bins[c2][j]] - deg[bins[c][i]]
                        load[c] = load[c] - dv
                        load[c2] = load[c2] + dv
                        moved += 1
            for c in cores:
                for s, n in enumerate(bins[c]):
                    perm[n] = c * OWN + b * P + s
    return perm


def _greedy_pack(nodes, deg_lo, deg_hi, cores, perm):
    """Quota-aware greedy packing of `nodes` into the bins of `cores`:
    penalize any bin-seg load crossing its chunk quota (1024 full / 896
    last block) hard, tiebreak on min-max load."""
    cores = list(cores)
    order = nodes[np.argsort(-(deg_lo[nodes] + deg_hi[nodes]), kind="stable")]
    nbins = len(cores) * NBLK
    cap = np.zeros(nbins, dtype=np.int64)
    load = np.zeros((nbins, 2), dtype=np.float64)
    capacity = np.full(nbins, P, dtype=np.int64)
    capacity[NBLK - 1 :: NBLK] = LAST_ROWS
    quota = np.full(nbins, 2048.0)
    quota[NBLK - 1 :: NBLK] = 1792.0
    full = np.zeros(nbins, dtype=bool)
    for n in order:
        l0 = load[:, 0] + deg_lo[n]
        l1 = load[:, 1] + deg_hi[n]
        over = np.maximum(l0 - quota, 0.0) + np.maximum(l1 - quota, 0.0)
        score = over * 1e9 + np.maximum(l0, l1)
        score[full] = np.inf
        b = int(np.argmin(score))
        c, blk = cores[b // NBLK], b % NBLK
        perm[n] = c * OWN + blk * P + cap[b]
        cap[b] += 1
        load[b, 0] += deg_lo[n]
        load[b, 1] += deg_hi[n]
        if cap[b] >= capacity[b]:
            full[b] = True
    return perm


DUMP_BLK = 0  # global overflow block: its chunk count may grow, all others stay at quota


def _shed_cross_block(perm, src, dst):
    """Per core, move overflow above each block's quota into DUMP_BLK (or
    under-quota blocks) via same-core swaps. Same core => same side =>
    segment membership of all edges is unchanged."""
    deg = np.zeros((N_NODES, 2), dtype=np.int64)
    np.add.at(deg, (dst, 0), 1)
    node_of = np.argsort(perm)
    quota = np.array([2048] * (NBLK - 1) + [1792])
    for c in range(N_CORES):
        rows = [LAST_ROWS if b == NBLK - 1 else P for b in range(NBLK)]
        bins = [
            list(node_of[c * OWN + b * P : c * OWN + b * P + rows[b]])
            for b in range(NBLK)
        ]
        load = np.array([[deg[bn, 0].sum(), deg[bn, 1].sum()] for bn in bins])
        for g in (0, 1):
            for b in range(NBLK):
                if b == DUMP_BLK:
                    continue
                guard = 0
                while load[b, g] > quota[b] and guard < 300:
                    guard += 1
                    # receiver: under-quota block with most room, else dump
                    room = quota - load[:, g]
                    room[b] = -1
                    r = int(np.argmax(room))
                    if room[r] <= 0:
                        r = DUMP_BLK
                    i = max(range(len(bins[b])), key=lambda i: deg[bins[b][i], g])
                    j = min(range(len(bins[r])), key=lambda j: deg[bins[r][j], g])
                    d = deg[bins[b][i], g] - deg[bins[r][j], g]
                    dgo = deg[bins[b][i], 1 - g] - deg[bins[r][j], 1 - g]
                    if d <= 0:
                        break
                    if r != DUMP_BLK and (
                        load[r, g] + d > quota[r]
                        or load[r, 1 - g] + dgo > quota[r]
                    ):
                        r = DUMP_BLK
                        j = min(
                            range(len(bins[r])), key=lambda j: deg[bins[r][j], g]
                        )
                        d = deg[bins[b][i], g] - deg[bins[r][j], g]
                        dgo = deg[bins[b][i], 1 - g] - deg[bins[r][j], 1 - g]
                        if d <= 0:
                            break
                    bins[b][i], bins[r][j] = bins[r][j], bins[b][i]
                    load[b, g] -= d
                    load[b, 1 - g] -= dgo
                    load[r, g] += d
                    load[r, 1 - g] += dgo
        for b in range(NBLK):
            for s, n in enumerate(bins[b]):
                perm[n] = c * OWN + b * P + s
    return perm


def balance_permutation(edge_index):
    """Relabel nodes so per-(core, block) incoming-edge counts are as
    uniform as possible (single global segment): quota-aware greedy
    packing against the 2048-edge/block chunk quota, then cross-block
    shedding + within-block swaps to equalize cores."""
    src = edge_index[0].astype(np.int64)
    dst = edge_index[1].astype(np.int64)
    tot = np.bincount(dst, minlength=N_NODES)
    perm = np.empty(N_NODES, np.int64)
    _greedy_pack(
        np.arange(N_NODES), tot, np.zeros_like(tot), range(N_CORES), perm
    )
    for _ in range(2):
        perm = _shed_cross_block(perm, src, dst)
        perm = _refine_swaps(perm, src, dst)
    return perm


def _chunk_plan(C3, g):
    """Phase-g chunk plan: list of (block, kind, col) with kind 0=pure-even,
    1=pure-odd, 2=mixed; col = first dstloc column of the chunk (mixed
    chunks own cols col, col+1). Shared by host packing and device
    emission so the layouts cannot drift apart."""
    plan = []
    col = 0
    for b in range(NBLK):
        for _ in range(int(C3[g, 0, b])):
            plan.append((b, 0, col))
            col += 1
        for _ in range(int(C3[g, 1, b])):
            plan.append((b, 1, col))
            col += 1
        for _ in range(int(C3[g, 2, b])):
            plan.append((b, 2, col))
            col += 2
    return plan, col


def prep_graph(edge_index):
    """Partition edges by destination core, bucket by source segment,
    sort by destination block, pad to uniform per-(block, seg) chunk
    counts across cores. Returns per-core packed arrays + chunk plan.

    idx values are PAIR indices (src_local // 2); dstloc2 has two columns
    per chunk: col 2t = dst for even-parity sources (255 otherwise),
    col 2t+1 = dst for odd-parity sources."""
    src = edge_index[0].astype(np.int64)
    dst = edge_index[1].astype(np.int64)
    # degrees include the self-loops, but the self-loop contribution is
    # added locally in the epilogue (no gather needed for it)
    deg = 1.0 + np.bincount(dst, minlength=N_NODES).astype(np.float32)
    dis = (1.0 / np.sqrt(np.maximum(deg, 1.0))).astype(np.float32)

    nseg = 1
    segb = [0, N_NODES]

    # per (core, block) edge lists with GLOBAL source ids (single
    # segment: the whole node table is exchanged in one 8-wide
    # AllGather; global pair ids < 25024 stay int16-safe)
    lists = {}
    core_of = dst // OWN
    for c in range(N_CORES):
        m = core_of == c
        s_c = src[m]
        d_c = dst[m] - c * OWN
        blk = d_c // P
        for b in range(NBLK):
            mb = blk == b
            lists[(c, b, 0)] = (s_c[mb], d_c[mb] - b * P)

    # chunk plan per (seg, block): C[g] = [pure-even, pure-odd, mixed]
    # chunk counts, uniform across cores. Pure quotas are the min over
    # cores of floor(count/128) so every core fills them exactly; the
    # remainder (surplus even + surplus odd + padding) goes into mixed
    # chunks, whose count is chosen so the TOTAL equals the old
    # parity-agnostic quota: ceil((a+b)/128) == floor(a/128)+floor(b/128)
    # + ceil((ra+rb)/128), so no chunk inflation ever.
    C = np.zeros((nseg, 3, NBLK), dtype=np.int64)
    for g in range(nseg):
        for b in range(NBLK):
            ne = [int((lists[(c, b, g)][0] % 2 == 0).sum())
                  for c in range(N_CORES)]
            no = [int((lists[(c, b, g)][0] % 2 == 1).sum())
                  for c in range(N_CORES)]
            ctot = max(
                max((e + o + P - 1) // P for e, o in zip(ne, no)), 1
            )
            ce = min(e // P for e in ne)
            co = min(o // P for o in no)
            C[g, 0, b] = ce
            C[g, 1, b] = co
            C[g, 2, b] = ctot - ce - co

    # pack per core: idx per seg (chunk-plan order), dstloc columns per
    # _chunk_plan (1 col per pure chunk, 2 per mixed)
    per_core = []
    for c in range(N_CORES):
        idx_segs = []
        dl_parts = []
        for g in range(nseg):
            flat = []
            for b in range(NBLK):
                s_l, d_l = lists[(c, b, g)]
                pe = (s_l % 2) == 0
                se, de = s_l[pe], d_l[pe]
                so, do_ = s_l[~pe], d_l[~pe]
                ne = int(C[g, 0, b]) * P
                no = int(C[g, 1, b]) * P
                ms = np.concatenate([se[ne:], so[no:]]) // 2
                md = np.concatenate([de[ne:], do_[no:]]).astype(np.float32)
                mp = np.concatenate(
                    [np.zeros(len(se) - ne, np.int64),
                     np.ones(len(so) - no, np.int64)]
                )
                n_pad = int(C[g, 2, b]) * P - len(ms)
                flat += [se[:ne] // 2, so[:no] // 2, ms,
                         np.zeros(n_pad, dtype=np.int64)]
                dl_parts.append(de[:ne].astype(np.float32))
                dl_parts.append(do_[:no].astype(np.float32))
                mdp = np.concatenate([md, np.full(n_pad, 255.0, np.float32)])
                mpp = np.concatenate([mp, np.full(n_pad, -1, np.int64)])
                for t in range(int(C[g, 2, b])):
                    sd = mdp[t * P : (t + 1) * P]
                    sp = mpp[t * P : (t + 1) * P]
                    dl_parts.append(np.where(sp == 0, sd, 255.0))
                    dl_parts.append(np.where(sp == 1, sd, 255.0))
            idx_segs.append(_wrap_idx(np.concatenate(flat)))
        dl = np.concatenate(dl_parts).reshape(-1, P)  # [n_cols, P]
        dstloc = np.ascontiguousarray(dl.T).astype(BF16)  # [128, n_cols]
        per_core.append((idx_segs, dstloc))

    return dis, segb, C, per_core


def golden_aggregate(table, idx_segs, dstloc, segb, C):
    """Numpy mirror of the on-device aggregation (for packing validation).
    table: [N_NODES, F_TAB] (host dtype); returns agg [NBLK*P, F_TAB]."""
    nseg = C.shape[0]
    tab = np.asarray(table, np.float32)
    tabp = np.concatenate([tab, np.zeros((48, F_TAB), np.float32)])
    pairs = tabp.reshape(-1, 2 * F_TAB)  # [25024, 128]
    agg = np.zeros((NBLK * P, F_TAB), np.float32)
    dl = np.asarray(dstloc, np.float32)
    t_ph = 0
    for g in range(nseg):
        w = idx_segs[g][:16, :]  # [16, T*8]
        flat_idx = w.T.reshape(-1)  # idx j at [j%16, j//16]
        seg = pairs[segb[g] // 2 :]
        t0 = 0
        for b in range(NBLK):
            for t in range(C[g, b]):
                tg = t_ph + t0 + t  # global chunk col
                rows = seg[flat_idx[(t0 + t) * P : (t0 + t + 1) * P]]  # [128,128]
                iota = np.arange(P)[None, :]
                sel_e = (dl[:, 2 * tg][:, None] == iota).astype(np.float32)
                sel_o = (dl[:, 2 * tg + 1][:, None] == iota).astype(np.float32)
                agg[b * P : (b + 1) * P] += (
                    sel_e.T @ rows[:, :F_TAB] + sel_o.T @ rows[:, F_TAB:]
                )
            t0 += C[g, b]
        t_ph += t0
    return agg


def build_program(C, segb, t_tot):
    import concourse.bacc as bacc
    import concourse.mybir as mybir
    import concourse.tile as tile

    f32 = mybir.dt.float32
    bf16 = mybir.dt.bfloat16
    i16 = mybir.dt.int16
    nseg = C.shape[0]
    n_idx = [int(C[g].sum()) * 8 for g in range(nseg)]  # idx free dim per seg
    NPAIR = SEG_BOUND // 2  # 12500 pairs per segment

    nc = bacc.Bacc(
        "TRN2",
        num_devices=1 if SIM_1CORE else N_CORES,
        num_swdge_queues=N_QUEUES,
        dynamic_dma_scratch_size=DMA_SCRATCH,
    )

    # ---- I/O ----
    xT = nc.dram_tensor("xT", [F_IN, OWN], bf16, kind="ExternalInput")
    W1 = nc.dram_tensor("W1", [F_IN, F_TAB], bf16, kind="ExternalInput")
    b1 = nc.dram_tensor("b1", [P, F_TAB], f32, kind="ExternalInput")
    b2 = nc.dram_tensor("b2", [P, F_TAB], f32, kind="ExternalInput")
    b3 = nc.dram_tensor("b3", [P, F_TAB], f32, kind="ExternalInput")
    dis_own = nc.dram_tensor("dis_own", [P, NBLK], f32, kind="ExternalInput")
    # W2/W3 stacked for the pair-transposed lhsT (bf16): the xbar
    # transpose of a block pair puts block A's features on partitions
    # 0:64 and block B's on 64:128; ops at base partition 64 fault on
    # this HW, so each matmul contracts all 128 partitions against a
    # half-zeroed weight stack ([W;0] for slot 0, [0;W] for slot 1)
    W2r = nc.dram_tensor("W2r", [P, 2 * F_TAB], bf16, kind="ExternalInput")
    W3r = nc.dram_tensor("W3r", [P, 2 * F_TAB], bf16, kind="ExternalInput")
    dmat_d = nc.dram_tensor(
        "dmat", [P, P * 2 * G_SEL], bf16, kind="ExternalInput"
    )
    idx_d = [
        nc.dram_tensor(f"idx{g}", [P, n_idx[g]], i16, kind="ExternalInput")
        for g in range(nseg)
    ]
    n_cols = sum(_chunk_plan(C, g)[1] for g in range(nseg))
    dstloc_d = nc.dram_tensor("dstloc", [P, n_cols], bf16, kind="ExternalInput")
    out = nc.dram_tensor("out", [OWN, F_OUT], f32, kind="ExternalOutput")

    # ---- internal DRAM ----
    ag_in = nc.dram_tensor("ag_in", [OWN, F_TAB], bf16)
    # tabF[l]: the full node table (single 8-wide AllGather output,
    # node order == core order so the concat lands directly in layout)
    tabF = [
        nc.dram_tensor(f"tabF{l}", [N_NODES + 48, F_TAB], bf16)
        for l in range(3)
    ]

    bias_d = [b1, b2, b3]

    with tile.TileContext(nc) as tc:
        with (
            tc.tile_pool(name="const", bufs=1) as const_pool,
            tc.tile_pool(name="xt", bufs=1) as xt_pool,
            tc.tile_pool(name="hbuf", bufs=1) as h_pool,
            tc.tile_pool(name="hrelu", bufs=2) as hr_pool,
            tc.tile_pool(name="msg", bufs=MSG_BUFS) as msg_pool,
            tc.tile_pool(name="sel", bufs=3) as sel_pool,
            tc.tile_pool(name="small", bufs=3) as small_pool,
        ):
            # ---- preload constants ----
            dmat_sb = const_pool.tile([P, P, 2 * G_SEL], bf16, tag="dmat")
            nc.sync.dma_start(
                out=dmat_sb[:].rearrange("p a b -> p (a b)"), in_=dmat_d[:]
            )
            dis_sb = const_pool.tile([P, NBLK], f32, tag="dis")
            nc.sync.dma_start(out=dis_sb[:], in_=dis_own[:])
            W1_sb = const_pool.tile([F_IN, F_TAB], bf16, tag="w0")
            nc.sync.dma_start(out=W1_sb[:], in_=W1[:])
            bias_sb = []
            for l in range(3):
                b_t = const_pool.tile([P, F_TAB], f32, tag=f"b{l}")
                nc.sync.dma_start(out=b_t[:], in_=bias_d[l][:])
                bias_sb.append(b_t)
            Wr_sb = []
            for i, wd in enumerate((W2r, W3r)):
                w_t = const_pool.tile([P, 2, F_TAB], bf16, tag=f"wr{i}")
                nc.sync.dma_start(
                    out=w_t[:].rearrange("p a b -> p (a b)"), in_=wd[:]
                )
                Wr_sb.append(w_t)
            idx_sb = []
            for g in range(nseg):
                t_i = const_pool.tile([P, n_idx[g]], i16, tag=f"idx{g}")
                nc.sync.dma_start(out=t_i[:], in_=idx_d[g][:])
                idx_sb.append(t_i)
            dstloc_sb = const_pool.tile([P, n_cols], bf16, tag="dstloc")
            nc.sync.dma_start(out=dstloc_sb[:], in_=dstloc_d[:])
            xt_sb = xt_pool.tile([F_IN, NBLK * P], bf16, tag="xt")
            if OWN < NBLK * P:
                nc.vector.memset(xt_sb[:, OWN:], 0.0)
            nc.sync.dma_start(out=xt_sb[:, :OWN], in_=xT[:])

            def emit_halo_stage1(l):
                # single 8-wide AllGather: ag_in (this core's shard) ->
                # tabF[l] (the whole node table)
                if SIM_1CORE:
                    nc.sync.dma_start(out=tabF[l][:OWN, :], in_=ag_in[:])
                elif not SKIP_AG:
                    nc.gpsimd.collective_compute(
                        "AllGather",
                        mybir.AluOpType.bypass,
                        replica_groups=[[0, 1, 2, 3, 4, 5, 6, 7]],
                        ins=[ag_in[:].opt()],
                        outs=[tabF[l][:N_NODES, :].opt()],
                    )

            t_own = None  # [P, NBLK, F_TAB] f32; layer l's comes from l-1
            for l in range(L_MAX):
                if l == 0:
                    # ======== layer-0 transform from xT ============
                    t_own = h_pool.tile(
                        [P, NBLK, F_TAB], f32, tag="town", name="town_0"
                    )
                    agp = h_pool.tile(
                        [P, NBLK, F_TAB], bf16, tag="agp", name="agp_0"
                    )
                    with tc.tile_pool(name="tf", bufs=2, space="PSUM") as tf_pool:
                        for k in range(N_BANK):
                            nb = min(8, NBLK - 8 * k)
                            mmps = tf_pool.tile([P, 8, F_TAB], f32, tag="mm")
                            for j in range(nb):
                                b = 8 * k + j
                                nc.tensor.matmul(
                                    out=mmps[:, j, :],
                                    lhsT=xt_sb[:, b * P : (b + 1) * P],
                                    rhs=W1_sb[:],
                                    start=True,
                                    stop=True,
                                )
                            # batched scale by dis (per-block per-partition)
                            nc.vector.tensor_tensor(
                                out=t_own[:, 8 * k : 8 * k + nb, :],
                                in0=mmps[:, :nb, :],
                                in1=dis_sb[
                                    :, 8 * k : 8 * k + nb, None
                                ].to_broadcast([P, nb, F_TAB]),
                                op=mybir.AluOpType.mult,
                            )
                            nc.scalar.copy(
                                out=agp[:, 8 * k : 8 * k + nb, :],
                                in_=t_own[:, 8 * k : 8 * k + nb, :],
                            )
                    # packed shard -> ag_in (full blocks + tail)
                    nc.sync.dma_start(
                        out=ag_in[: (NBLK - 1) * P, :]
                        .rearrange("(b p) f -> p b f", p=P),
                        in_=agp[:, : NBLK - 1, :],
                    )
                    nc.sync.dma_start(
                        out=ag_in[(NBLK - 1) * P :, :]
                        .rearrange("(b p) f -> p b f", p=LAST_ROWS),
                        in_=agp[:LAST_ROWS, NBLK - 1 : NBLK, :],
                    )
                    emit_halo_stage1(0)
                # layers 1/2: t_own, ag_in and the stage-1 AG were all
                # produced inside layer l-1's aggregation (per-bank tails)
                # pair view: [*, 128] bf16 (256B rows)
                pairsF = tabF[l][:].rearrange("(a b) c -> a (b c)", b=2)
                # ======== aggregate into per-block PSUM ========
                with tc.tile_pool(name="banks", bufs=1, space="PSUM") as bk_pool:
                    banks = [
                        bk_pool.tile(
                            [P, 8, F_TAB], f32, tag=f"bank{k}", name=f"bank{k}_{l}"
                        )
                        for k in range(N_BANK)
                    ]
                    # per-layer epilogue + next-layer transform state
                    hsum = h_pool.tile(
                        [P, NBLK, F_TAB], f32, tag="hsum", name=f"hsum_{l}"
                    )
                    if l < L_MAX - 1:
                        # +1 pad block so DMA transposes batch in pairs
                        h_next = hr_pool.tile(
                            [P, NBLK + 1, F_TAB], bf16, tag="h",
                            name=f"h_{l}",
                        )
                        nc.vector.memset(h_next[:, NBLK:, :], 0.0)
                        t_next = h_pool.tile(
                            [P, NBLK, F_TAB], f32, tag="town",
                            name=f"town_{l + 1}",
                        )
                        agp_next = h_pool.tile(
                            [P, NBLK, F_TAB], bf16, tag="agp",
                            name=f"agp_{l + 1}",
                        )

                    def emit_tail(k, l=l):
                        """Epilogue for PSUM bank k; for l<2 also the
                        next-layer transform of its blocks + ag_in write.
                        Runs inside the seg-1 emission as each bank's last
                        accumulation chain closes, overlapping the tail
                        with the remaining gather-bound aggregation."""
                        nb = min(8, NBLK - 8 * k)
                        sl = slice(8 * k, 8 * k + nb)
                        nc.vector.tensor_tensor(
                            out=hsum[:, sl, :],
                            in0=banks[k][:, :nb, :],
                            in1=t_own[:, sl, :],
                            op=mybir.AluOpType.add,
                        )
                        nc.vector.tensor_tensor(
                            out=hsum[:, sl, :],
                            in0=hsum[:, sl, :],
                            in1=dis_sb[:, sl, None].to_broadcast(
                                [P, nb, F_TAB]
                            ),
                            op=mybir.AluOpType.mult,
                        )
                        nc.vector.tensor_tensor(
                            out=hsum[:, sl, :],
                            in0=hsum[:, sl, :],
                            in1=bias_sb[l][:, None, :].to_broadcast(
                                [P, nb, F_TAB]
                            ),
                            op=mybir.AluOpType.add,
                        )
                        if l == L_MAX - 1:
                            if k < N_BANK - 1:
                                nc.sync.dma_start(
                                    out=out[8 * k * P : (8 * k + nb) * P, :]
                                    .rearrange("(b p) f -> p b f", p=P),
                                    in_=hsum[:, sl, :F_OUT],
                                )
                            else:
                                nc.sync.dma_start(
                                    out=out[(NBLK - 1) * P :, :]
                                    .rearrange(
                                        "(b p) f -> p b f", p=LAST_ROWS
                                    ),
                                    in_=hsum[
                                        :LAST_ROWS, NBLK - 1 : NBLK, :F_OUT
                                    ],
                                )
                            return
                        nc.vector.tensor_scalar(
                            out=h_next[:, sl, :],
                            in0=hsum[:, sl, :],
                            scalar1=0.0,
                            scalar2=None,
                            op0=mybir.AluOpType.max,
                        )
                        # transform bank k for layer l+1: bf16 DMA
                        # transposes (2 blocks/xbar pass), matmuls reuse
                        # bank k's freshly-drained PSUM region
                        mmps = bk_pool.tile(
                            [P, 8, F_TAB], f32, tag=f"bank{k}",
                            name=f"mm{k}_{l}",
                        )
                        for q in range(0, nb, 2):
                            tr2 = small_pool.tile([P, P], bf16, tag="tr2")
                            nc.sync.dma_start_transpose(
                                out=tr2[:],
                                in_=h_next[:, 8 * k + q : 8 * k + q + 2, :],
                            )
                            for jj in range(2):
                                b2 = 8 * k + q + jj
                                if b2 >= NBLK:
                                    continue
                                nc.tensor.matmul(
                                    out=mmps[:, q + jj, :],
                                    lhsT=tr2[:, :],
                                    rhs=Wr_sb[l][:, jj, :],
                                    start=True,
                                    stop=True,
                                )
                        nc.vector.tensor_tensor(
                            out=t_next[:, sl, :],
                            in0=mmps[:, :nb, :],
                            in1=dis_sb[:, sl, None].to_broadcast(
                                [P, nb, F_TAB]
                            ),
                            op=mybir.AluOpType.mult,
                        )
                        nc.scalar.copy(
                            out=agp_next[:, sl, :],
                            in_=t_next[:, sl, :],
                        )
                        if k < N_BANK - 1:
                            nc.sync.dma_start(
                                out=ag_in[8 * k * P : (8 * k + nb) * P, :]
                                .rearrange("(b p) f -> p b f", p=P),
                                in_=agp_next[:, sl, :],
                            )
                        else:
                            nc.sync.dma_start(
                                out=ag_in[(NBLK - 1) * P :, :]
                                .rearrange("(b p) f -> p b f", p=LAST_ROWS),
                                in_=agp_next[
                                    :LAST_ROWS, NBLK - 1 : NBLK, :
                                ],
                            )
                            # NOTE: banks k<6 cover 8 full blocks; only the
                            # last bank holds the short tail block

                    agg_lo = None
                    col_base = 0
                    call_no = 0
                    for g in range(nseg):
                        in_view = pairsF[: N_NODES // 2 + 12, :]
                        plan, ncols_g = _chunk_plan(C, g)
                        n_ch = len(plan)
                        # NOTE: accumulation chains to one PSUM region must
                        # stay contiguous in PE order — interleaving open
                        # chains (deferred stop) corrupts results on HW —
                        # so each segment closes its chains and seg-0
                        # partials are evacuated to SBUF below.
                        tot_b = C[g].sum(axis=0)  # total chunks per block
                        ci = np.zeros(NBLK, dtype=np.int64)
                        for call0 in range(0, n_ch, G_CALL):
                            gcnt = min(G_CALL, n_ch - call0)
                            msg = msg_pool.tile(
                                [P, G_CALL, 2 * F_TAB], bf16, tag="msg"
                            )
                            geff = 1 if GATHER_OFF else gcnt
                            nc.gpsimd.dma_gather(
                                out_ap=msg[:, :geff, :],
                                in_ap=in_view,
                                idxs_ap=idx_sb[g][
                                    :, call0 * 8 : (call0 + geff) * 8
                                ],
                                num_idxs=geff * P,
                                num_idxs_reg=geff * P,
                                elem_size=2 * F_TAB,
                                single_packet=SINGLE_PACKET,
                                queue_num=call_no % N_QUEUES,
                            )
                            call_no += 1
                            for s0 in range(0, gcnt, G_SEL):
                                scnt = min(G_SEL, gcnt - s0)
                                first = plan[call0 + s0]
                                lastc = plan[call0 + s0 + scnt - 1]
                                col0 = first[2]
                                ncols = (
                                    lastc[2] + (2 if lastc[1] == 2 else 1)
                                    - col0
                                )
                                # sel layout [edge, dst, chunkcol]: all
                                # operands keep a real (stride-1, >=2)
                                # last dim, so the DVE runs this in the
                                # 2x_1p fast mode (a last-dim-broadcast
                                # operand would force 1x)
                                sel = sel_pool.tile(
                                    [P, P, 2 * G_SEL], bf16, tag="sel"
                                )
                                nc.vector.tensor_tensor(
                                    out=sel[:, :, :ncols],
                                    in0=dstloc_sb[
                                        :,
                                        None,
                                        col_base + col0 : col_base + col0
                                        + ncols,
                                    ].to_broadcast([P, P, ncols]),
                                    in1=dmat_sb[:, :, :ncols],
                                    op=mybir.AluOpType.is_equal,
                                )
                                for j in range(scnt):
                                    b, kind, colc = plan[call0 + s0 + j]
                                    lc = colc - col0
                                    last = bool(ci[b] == tot_b[b] - 1)
                                    if kind < 2:
                                        nc.tensor.matmul(
                                            out=banks[b // 8][:, b % 8, :],
                                            lhsT=sel[:, :, lc],
                                            rhs=msg[
                                                :,
                                                s0 + j,
                                                kind * F_TAB
                                                : (kind + 1) * F_TAB,
                                            ],
                                            start=bool(ci[b] == 0),
                                            stop=last,
                                        )
                                    else:
                                        nc.tensor.matmul(
                                            out=banks[b // 8][:, b % 8, :],
                                            lhsT=sel[:, :, lc],
                                            rhs=msg[:, s0 + j, 0:F_TAB],
                                            start=bool(ci[b] == 0),
                                            stop=False,
                                        )
                                        nc.tensor.matmul(
                                            out=banks[b // 8][:, b % 8, :],
                                            lhsT=sel[:, :, lc + 1],
                                            rhs=msg[
                                                :, s0 + j, F_TAB : 2 * F_TAB
                                            ],
                                            start=False,
                                            stop=last,
                                        )
                                    ci[b] += 1
                                    if (
                                        last
                                        and g == nseg - 1
                                        and b == min(8 * (b // 8) + 7,
                                                     NBLK - 1)
                                    ):
                                        # bank b//8's chains all closed (the
                                        # plan is block-ordered); no chain
                                        # is open right here, so the tail's
                                        # standalone matmuls can't split one
                                        emit_tail(b // 8)
                        col_base += ncols_g
                        if nseg == 2 and g == 0:
                            # evacuate first-phase partials to reuse banks
                            agg_lo = h_pool.tile(
                                [P, NBLK, F_TAB], f32, tag="agglo",
                                name=f"agglo_{l}",
                            )
                            for k in range(N_BANK):
                                nb = min(8, NBLK - 8 * k)
                                nc.scalar.copy(
                                    out=agg_lo[:, 8 * k : 8 * k + nb, :],
                                    in_=banks[k][:, :nb, :],
                                )
                    # ======== next layer's own-side exchange ========
                    if l < L_MAX - 1:
                        emit_halo_stage1(l + 1)
                        t_own = t_next

    nc.compile()
    return nc


def _stack_w(W):
    """[P, 2, F_TAB] bf16: slot 0 = [W; 0], slot 1 = [0; W] (see W2r)."""
    z = np.zeros((F_TAB, F_TAB), np.float32)
    a = np.concatenate([W, z], axis=0)  # [128, 64]
    b = np.concatenate([z, W], axis=0)
    return np.ascontiguousarray(
        np.stack([a, b], axis=1).reshape(P, 2 * F_TAB)
    ).astype(BF16)


def make_in_maps(x, W1, b1, W2, b2, W3, b3, dis, C, per_core):
    W3p = np.zeros((F_TAB, F_TAB), np.float32)
    W3p[:, :F_OUT] = np.asarray(W3, np.float32)
    b3p = np.zeros((F_TAB,), np.float32)
    b3p[:F_OUT] = np.asarray(b3, np.float32)
    # dmat[r, d, c] = d  (real stride-1 last dim for the DVE 2x sel build)
    dmat = np.broadcast_to(
        np.arange(P, dtype=np.float32)[None, :, None], (P, P, 2 * G_SEL)
    ).reshape(P, -1).astype(BF16).copy()

    in_maps = []
    for c in range(N_CORES):
        idx_segs, dstloc = per_core[c]
        d_own = dis[c * OWN : (c + 1) * OWN]
        pad = np.concatenate([d_own, np.ones(NBLK * P - OWN, np.float32)])
        m = {
            "xT": np.ascontiguousarray(
                x[c * OWN : (c + 1) * OWN].T
            ).astype(BF16),
            "W1": np.asarray(W1, np.float32).astype(BF16),
            "b1": np.broadcast_to(np.asarray(b1, np.float32), (P, F_TAB)).copy(),
            "b2": np.broadcast_to(np.asarray(b2, np.float32), (P, F_TAB)).copy(),
            "b3": np.broadcast_to(b3p, (P, F_TAB)).copy(),
            "dis_own": np.ascontiguousarray(pad.reshape(NBLK, P).T),
            "W2r": _stack_w(np.asarray(W2, np.float32)),
            "W3r": _stack_w(W3p),
            "dmat": dmat,
            "dstloc": dstloc,
        }
        for g in range(C.shape[0]):
            m[f"idx{g}"] = idx_segs[g]
        in_maps.append(m)
    return in_maps


_CACHE = {}


def kernel(x, edge_index, W1, b1, W2, b2, W3, b3):
    from concourse import bass_utils

    x = np.asarray(x, dtype=np.float32)
    edge_index = np.asarray(edge_index)
    key = hash(edge_index.tobytes())
    if key in _CACHE:
        nc, dis, segb, C, per_core, perm = _CACHE[key]
    else:
        perm = balance_permutation(edge_index)
        edge_perm = perm[np.asarray(edge_index, dtype=np.int64)]
        dis, segb, C, per_core = prep_graph(edge_perm)
        nc = build_program(C, segb, int(C.sum()))
        _CACHE[key] = (nc, dis, segb, C, per_core, perm)
    inv = np.argsort(perm)
    x = x[inv]  # x in new-id row order

    in_maps = make_in_maps(x, W1, b1, W2, b2, W3, b3, dis, C, per_core)

    res = bass_utils.run_bass_kernel_spmd(
        nc, in_maps, core_ids=list(range(N_CORES))
    )
    out = np.concatenate([res.results[c]["out"] for c in range(N_CORES)], axis=0)
    return out[perm]  # back to original node order



# revision 48
# speedup vs baseline: 1.1076x; 1.1076x over previous
"""3-layer GCN (GCNConv x3) on 8 Trainium2 NeuronCores.

Strategy (node-partitioned, PE scatter-add), V4:
  - Nodes are block-partitioned across the 8 cores by destination id
    (core c owns rows [c*OWN, (c+1)*OWN)).
  - Per layer: each core transforms its own node rows (H @ W, scaled by
    dis = deg^-1/2) into a packed bf16 table shard; ONE 8-wide
    AllGather per layer lands the full node table in each core's DRAM
    (node order == core order, and global pair ids < 25024 fit int16,
    so no segmenting is needed); then each core aggregates messages
    for its own destinations:
      * the table is gathered PAIR-wise: dma_gather needs >=256B
        descriptors, and a bf16 row is only 128B, so each descriptor
        fetches the 256B pair (h[2i], h[2i+1]) and the aggregation
        selects the correct half per edge,
      * scaled+merged into per-destination-block PSUM accumulators via
        selection-matrix matmuls on the tensor engine (even-source
        edges contract against msg cols 0:64, odd-source edges against
        cols 64:128); sel matrices are built by a batched DVE is_equal
        in [edge, dst, chunk] layout against a materialized index
        constant so every operand keeps a real stride-1 last dim and
        the DVE runs in its 2x fast mode,
      * each block's PSUM accumulation chain is one contiguous run in
        PE program order (interleaved open chains, or reopening a
        closed chain with start=False, corrupt results on this HW; the
        single-segment plan keeps every chain contiguous for free).
  - Tail pipelining: as each PSUM bank's chain closes, its epilogue
    (+ t_own, * dis[dst], + bias, relu -> bf16), the NEXT layer's
    transform of those blocks (xbar DMA pair-transposes + bf16 matmuls
    reusing the just-drained PSUM bank), and the bank's ag_in write
    are emitted inline, overlapping the gather-bound remainder of the
    aggregation; only the per-layer AllGather stays exposed.
  - norm = dis[src]*dis[dst] is folded as: table rows pre-scaled by
    dis[src], aggregated output post-scaled by dis[dst]; no per-edge
    scaling needed.

The host side only does graph partitioning / index packing (sorting,
bucketing, degree counts); all tensor math runs on the NeuronCores.
"""

import sys

sys.path.insert(0, "/opt/trn_rl_repo")

import numpy as np
import ml_dtypes

N_NODES = 50000
N_CORES = 8
OWN = N_NODES // N_CORES  # 6250
P = 128
NBLK = (OWN + P - 1) // P  # 49
LAST_ROWS = OWN - (NBLK - 1) * P  # 106
F_IN = 128
F_TAB = 64  # table width, all layers (layer-3 W padded 32->64)
F_OUT = 32
SEG_BOUND = 25000  # int16-safe source bucketing (pair ids < 12500)
G_SEL = 16  # chunks per DVE sel-build op
G_CALL = 48  # chunks per dma_gather call
N_BANK = (NBLK + 7) // 8  # PSUM banks for aggregation (7)
SINGLE_PACKET = False  # single-packet mode hangs SDMA on >1k-descriptor gathers
SKIP_AG = False  # ablation: drop the AllGather collectives
N_QUEUES = 4  # SWDGE queues; gather calls round-robin across them
DMA_SCRATCH = 16384  # dynamic-DMA scratch bytes (ring = /16 descriptors)
MSG_BUFS = 3  # in-flight gather destination tiles
SIM_1CORE = False  # build single-core (no collective) variant for TimelineSim
GATHER_OFF = False  # ablation: skip dma_gather calls (timing only, wrong result)
L_MAX = 3  # ablation: number of GCN layers to build (3 = full model)

BF16 = ml_dtypes.bfloat16


def _wrap_idx(flat_idx):
    """int16 gather-index layout: idx j at [j%16, j//16], replicated x8."""
    w = flat_idx.astype(np.int16).reshape(-1, 16).T  # [16, T*8]
    return np.ascontiguousarray(np.tile(w, (8, 1)))  # [128, T*8]


def _greedy_assign(deg_lo, deg_hi):
    """Greedy vector bin-packing of nodes into (core, block) bins by
    per-segment incoming-edge load. Returns perm (old id -> new id)."""
    tot = deg_lo + deg_hi
    order = np.argsort(-tot, kind="stable")
    nbins = N_CORES * NBLK
    cap = np.zeros(nbins, dtype=np.int64)
    load = np.zeros((nbins, 2), dtype=np.float64)
    capacity = np.full(nbins, P, dtype=np.int64)
    capacity[NBLK - 1 :: NBLK] = LAST_ROWS
    perm = np.empty(N_NODES, dtype=np.int64)
    score = load.max(axis=1) * 4096 + load.sum(axis=1)
    for n in order:
        b = int(np.argmin(score))
        c, blk = b // NBLK, b % NBLK
        perm[n] = c * OWN + blk * P + cap[b]
        cap[b] += 1
        load[b, 0] += deg_lo[n]
        load[b, 1] += deg_hi[n]
        if cap[b] >= capacity[b]:
            score[b] = np.inf
        else:
            score[b] = load[b].max() * 4096 + load[b].sum()
    return perm


def _refine_swaps(perm, src, dst):
    """Swap nodes between cores within the same block (and same core-side,
    so source-segment membership is unchanged) to pull every core's
    per-(block, seg) load under the block's ceil target."""
    deg = np.zeros((N_NODES, 2), dtype=np.int64)
    np.add.at(deg, (dst, 0), 1)
    node_of = np.argsort(perm)  # new id -> old node
    moved = 0
    for cores in (range(N_CORES),):
        for b in range(NBLK):
            rows = LAST_ROWS if b == NBLK - 1 else P
            bins = {
                c: list(node_of[c * OWN + b * P : c * OWN + b * P + rows])
                for c in cores
            }
            load = {
                c: np.array([deg[bins[c], 0].sum(), deg[bins[c], 1].sum()])
                for c in cores
            }
            for g in (0, 1):
                tgt = 128 * int(
                    np.ceil(np.mean([load[c][g] for c in cores]) / P)
                )
                for c in cores:
                    guard = 0
                    while load[c][g] > tgt and guard < 200:
                        guard += 1
                        c2 = min(cores, key=lambda k: load[k][g])
                        if load[c2][g] >= tgt:
                            break
                        need = load[c][g] - tgt
                        room = tgt - load[c2][g]
                        # donor: largest deg_g node; receiver: smallest
                        i = max(range(len(bins[c])), key=lambda i: deg[bins[c][i], g])
                        j = min(range(len(bins[c2])), key=lambda j: deg[bins[c2][j], g])
                        d = deg[bins[c][i], g] - deg[bins[c2][j], g]
                        if d <= 0 or d > need + room:
                            break
                        bins[c][i], bins[c2][j] = bins[c2][j], bins[c][i]
                        dv = deg[bins[c2][j]] - deg[bins[c][i]]
                        load[c] = load[c] - dv
                        load[c2] = load[c2] + dv
                        moved += 1
            for c in cores:
                for s, n in enumerate(bins[c]):
                    perm[n] = c * OWN + b * P + s
    return perm


def _greedy_pack(nodes, deg_lo, deg_hi, cores, perm):
    """Quota-aware greedy packing of `nodes` into the bins of `cores`:
    penalize any bin-seg load crossing its chunk quota (1024 full / 896
    last block) hard, tiebreak on min-max load."""
    cores = list(cores)
    order = nodes[np.argsort(-(deg_lo[nodes] + deg_hi[nodes]), kind="stable")]
    nbins = len(cores) * NBLK
    cap = np.zeros(nbins, dtype=np.int64)
    load = np.zeros((nbins, 2), dtype=np.float64)
    capacity = np.full(nbins, P, dtype=np.int64)
    capacity[NBLK - 1 :: NBLK] = LAST_ROWS
    quota = np.full(nbins, 2048.0)
    quota[NBLK - 1 :: NBLK] = 1792.0
    full = np.zeros(nbins, dtype=bool)
    for n in order:
        l0 = load[:, 0] + deg_lo[n]
        l1 = load[:, 1] + deg_hi[n]
        over = np.maximum(l0 - quota, 0.0) + np.maximum(l1 - quota, 0.0)
        score = over * 1e9 + np.maximum(l0, l1)
        score[full] = np.inf
        b = int(np.argmin(score))
        c, blk = cores[b // NBLK], b % NBLK
        perm[n] = c * OWN + blk * P + cap[b]
        cap[b] += 1
        load[b, 0] += deg_lo[n]
        load[b, 1] += deg_hi[n]
        if cap[b] >= capacity[b]:
            full[b] = True
    return perm


DUMP_BLK = 0  # global overflow block: its chunk count may grow, all others stay at quota


def _shed_cross_block(perm, src, dst):
    """Per core, move overflow above each block's quota into DUMP_BLK (or
    under-quota blocks) via same-core swaps. Same core => same side =>
    segment membership of all edges is unchanged."""
    deg = np.zeros((N_NODES, 2), dtype=np.int64)
    np.add.at(deg, (dst, 0), 1)
    node_of = np.argsort(perm)
    quota = np.array([2048] * (NBLK - 1) + [1792])
    for c in range(N_CORES):
        rows = [LAST_ROWS if b == NBLK - 1 else P for b in range(NBLK)]
        bins = [
            list(node_of[c * OWN + b * P : c * OWN + b * P + rows[b]])
            for b in range(NBLK)
        ]
        load = np.array([[deg[bn, 0].sum(), deg[bn, 1].sum()] for bn in bins])
        for g in (0, 1):
            for b in range(NBLK):
                if b == DUMP_BLK:
                    continue
                guard = 0
                while load[b, g] > quota[b] and guard < 300:
                    guard += 1
                    # receiver: under-quota block with most room, else dump
                    room = quota - load[:, g]
                    room[b] = -1
                    r = int(np.argmax(room))
                    if room[r] <= 0:
                        r = DUMP_BLK
                    i = max(range(len(bins[b])), key=lambda i: deg[bins[b][i], g])
                    j = min(range(len(bins[r])), key=lambda j: deg[bins[r][j], g])
                    d = deg[bins[b][i], g] - deg[bins[r][j], g]
                    dgo = deg[bins[b][i], 1 - g] - deg[bins[r][j], 1 - g]
                    if d <= 0:
                        break
                    if r != DUMP_BLK and (
                        load[r, g] + d > quota[r]
                        or load[r, 1 - g] + dgo > quota[r]
                    ):
                        r = DUMP_BLK
                        j = min(
                            range(len(bins[r])), key=lambda j: deg[bins[r][j], g]
                        )
                        d = deg[bins[b][i], g] - deg[bins[r][j], g]
                        dgo = deg[bins[b][i], 1 - g] - deg[bins[r][j], 1 - g]
                        if d <= 0:
                            break
                    bins[b][i], bins[r][j] = bins[r][j], bins[b][i]
                    load[b, g] -= d
                    load[b, 1 - g] -= dgo
                    load[r, g] += d
                    load[r, 1 - g] += dgo
        for b in range(NBLK):
            for s, n in enumerate(bins[b]):
                perm[n] = c * OWN + b * P + s
    return perm


def balance_permutation(edge_index):
    """Relabel nodes so per-(core, block) incoming-edge counts are as
    uniform as possible (single global segment): quota-aware greedy
    packing against the 2048-edge/block chunk quota, then cross-block
    shedding + within-block swaps to equalize cores."""
    src = edge_index[0].astype(np.int64)
    dst = edge_index[1].astype(np.int64)
    tot = np.bincount(dst, minlength=N_NODES)
    perm = np.empty(N_NODES, np.int64)
    _greedy_pack(
        np.arange(N_NODES), tot, np.zeros_like(tot), range(N_CORES), perm
    )
    for _ in range(2):
        perm = _shed_cross_block(perm, src, dst)
        perm = _refine_swaps(perm, src, dst)
    return perm


def _chunk_plan(C3, g):
    """Phase-g chunk plan: list of (block, kind, col) with kind 0=pure-even,
    1=pure-odd, 2=mixed; col = first dstloc column of the chunk (mixed
    chunks own cols col, col+1). Shared by host packing and device
    emission so the layouts cannot drift apart."""
    plan = []
    col = 0
    for b in range(NBLK):
        for _ in range(int(C3[g, 0, b])):
            plan.append((b, 0, col))
            col += 1
        for _ in range(int(C3[g, 1, b])):
            plan.append((b, 1, col))
            col += 1
        for _ in range(int(C3[g, 2, b])):
            plan.append((b, 2, col))
            col += 2
    return plan, col


def prep_graph(edge_index):
    """Partition edges by destination core, bucket by source segment,
    sort by destination block, pad to uniform per-(block, seg) chunk
    counts across cores. Returns per-core packed arrays + chunk plan.

    idx values are PAIR indices (src_local // 2); dstloc2 has two columns
    per chunk: col 2t = dst for even-parity sources (255 otherwise),
    col 2t+1 = dst for odd-parity sources."""
    src = edge_index[0].astype(np.int64)
    dst = edge_index[1].astype(np.int64)
    # degrees include the self-loops, but the self-loop contribution is
    # added locally in the epilogue (no gather needed for it)
    deg = 1.0 + np.bincount(dst, minlength=N_NODES).astype(np.float32)
    dis = (1.0 / np.sqrt(np.maximum(deg, 1.0))).astype(np.float32)

    nseg = 1
    segb = [0, N_NODES]

    # per (core, block) edge lists with GLOBAL source ids (single
    # segment: the whole node table is exchanged in one 8-wide
    # AllGather; global pair ids < 25024 stay int16-safe)
    lists = {}
    core_of = dst // OWN
    for c in range(N_CORES):
        m = core_of == c
        s_c = src[m]
        d_c = dst[m] - c * OWN
        blk = d_c // P
        for b in range(NBLK):
            mb = blk == b
            lists[(c, b, 0)] = (s_c[mb], d_c[mb] - b * P)

    # chunk plan per (seg, block): C[g] = [pure-even, pure-odd, mixed]
    # chunk counts, uniform across cores. Pure quotas are the min over
    # cores of floor(count/128) so every core fills them exactly; the
    # remainder (surplus even + surplus odd + padding) goes into mixed
    # chunks, whose count is chosen so the TOTAL equals the old
    # parity-agnostic quota: ceil((a+b)/128) == floor(a/128)+floor(b/128)
    # + ceil((ra+rb)/128), so no chunk inflation ever.
    C = np.zeros((nseg, 3, NBLK), dtype=np.int64)
    for g in range(nseg):
        for b in range(NBLK):
            ne = [int((lists[(c, b, g)][0] % 2 == 0).sum())
                  for c in range(N_CORES)]
            no = [int((lists[(c, b, g)][0] % 2 == 1).sum())
                  for c in range(N_CORES)]
            ctot = max(
                max((e + o + P - 1) // P for e, o in zip(ne, no)), 1
            )
            ce = min(e // P for e in ne)
            co = min(o // P for o in no)
            C[g, 0, b] = ce
            C[g, 1, b] = co
            C[g, 2, b] = ctot - ce - co

    # pack per core: idx per seg (chunk-plan order), dstloc columns per
    # _chunk_plan (1 col per pure chunk, 2 per mixed)
    per_core = []
    for c in range(N_CORES):
        idx_segs = []
        dl_parts = []
        for g in range(nseg):
            flat = []
            for b in range(NBLK):
                s_l, d_l = lists[(c, b, g)]
                pe = (s_l % 2) == 0
                se, de = s_l[pe], d_l[pe]
                so, do_ = s_l[~pe], d_l[~pe]
                # sort by table address: every chunk's 128 descriptors
                # then issue in ascending pair order (and chunks cover
                # disjoint ascending ranges), improving HBM row-buffer
                # locality of the random-access gathers
                oe = np.argsort(se // 2, kind="stable")
                se, de = se[oe], de[oe]
                oo = np.argsort(so // 2, kind="stable")
                so, do_ = so[oo], do_[oo]
                ne = int(C[g, 0, b]) * P
                no = int(C[g, 1, b]) * P
                ms = np.concatenate([se[ne:], so[no:]]) // 2
                md = np.concatenate([de[ne:], do_[no:]]).astype(np.float32)
                mp = np.concatenate(
                    [np.zeros(len(se) - ne, np.int64),
                     np.ones(len(so) - no, np.int64)]
                )
                om = np.argsort(ms, kind="stable")
                ms, md, mp = ms[om], md[om], mp[om]
                n_pad = int(C[g, 2, b]) * P - len(ms)
                flat += [se[:ne] // 2, so[:no] // 2, ms,
                         np.zeros(n_pad, dtype=np.int64)]
                dl_parts.append(de[:ne].astype(np.float32))
                dl_parts.append(do_[:no].astype(np.float32))
                mdp = np.concatenate([md, np.full(n_pad, 255.0, np.float32)])
                mpp = np.concatenate([mp, np.full(n_pad, -1, np.int64)])
                for t in range(int(C[g, 2, b])):
                    sd = mdp[t * P : (t + 1) * P]
                    sp = mpp[t * P : (t + 1) * P]
                    dl_parts.append(np.where(sp == 0, sd, 255.0))
                    dl_parts.append(np.where(sp == 1, sd, 255.0))
            idx_segs.append(_wrap_idx(np.concatenate(flat)))
        dl = np.concatenate(dl_parts).reshape(-1, P)  # [n_cols, P]
        dstloc = np.ascontiguousarray(dl.T).astype(BF16)  # [128, n_cols]
        per_core.append((idx_segs, dstloc))

    return dis, segb, C, per_core


def golden_aggregate(table, idx_segs, dstloc, segb, C):
    """Numpy mirror of the on-device aggregation (for packing validation).
    table: [N_NODES, F_TAB] (host dtype); returns agg [NBLK*P, F_TAB]."""
    nseg = C.shape[0]
    tab = np.asarray(table, np.float32)
    tabp = np.concatenate([tab, np.zeros((48, F_TAB), np.float32)])
    pairs = tabp.reshape(-1, 2 * F_TAB)  # [25024, 128]
    agg = np.zeros((NBLK * P, F_TAB), np.float32)
    dl = np.asarray(dstloc, np.float32)
    t_ph = 0
    for g in range(nseg):
        w = idx_segs[g][:16, :]  # [16, T*8]
        flat_idx = w.T.reshape(-1)  # idx j at [j%16, j//16]
        seg = pairs[segb[g] // 2 :]
        t0 = 0
        for b in range(NBLK):
            for t in range(C[g, b]):
                tg = t_ph + t0 + t  # global chunk col
                rows = seg[flat_idx[(t0 + t) * P : (t0 + t + 1) * P]]  # [128,128]
                iota = np.arange(P)[None, :]
                sel_e = (dl[:, 2 * tg][:, None] == iota).astype(np.float32)
                sel_o = (dl[:, 2 * tg + 1][:, None] == iota).astype(np.float32)
                agg[b * P : (b + 1) * P] += (
                    sel_e.T @ rows[:, :F_TAB] + sel_o.T @ rows[:, F_TAB:]
                )
            t0 += C[g, b]
        t_ph += t0
    return agg


def build_program(C, segb, t_tot):
    import concourse.bacc as bacc
    import concourse.mybir as mybir
    import concourse.tile as tile

    f32 = mybir.dt.float32
    bf16 = mybir.dt.bfloat16
    i16 = mybir.dt.int16
    nseg = C.shape[0]
    n_idx = [int(C[g].sum()) * 8 for g in range(nseg)]  # idx free dim per seg
    NPAIR = SEG_BOUND // 2  # 12500 pairs per segment

    nc = bacc.Bacc(
        "TRN2",
        num_devices=1 if SIM_1CORE else N_CORES,
        num_swdge_queues=N_QUEUES,
        dynamic_dma_scratch_size=DMA_SCRATCH,
    )

    # ---- I/O ----
    xT = nc.dram_tensor("xT", [F_IN, OWN], bf16, kind="ExternalInput")
    W1 = nc.dram_tensor("W1", [F_IN, F_TAB], bf16, kind="ExternalInput")
    b1 = nc.dram_tensor("b1", [P, F_TAB], f32, kind="ExternalInput")
    b2 = nc.dram_tensor("b2", [P, F_TAB], f32, kind="ExternalInput")
    b3 = nc.dram_tensor("b3", [P, F_TAB], f32, kind="ExternalInput")
    dis_own = nc.dram_tensor("dis_own", [P, NBLK], f32, kind="ExternalInput")
    # W2/W3 stacked for the pair-transposed lhsT (bf16): the xbar
    # transpose of a block pair puts block A's features on partitions
    # 0:64 and block B's on 64:128; ops at base partition 64 fault on
    # this HW, so each matmul contracts all 128 partitions against a
    # half-zeroed weight stack ([W;0] for slot 0, [0;W] for slot 1)
    W2r = nc.dram_tensor("W2r", [P, 2 * F_TAB], bf16, kind="ExternalInput")
    W3r = nc.dram_tensor("W3r", [P, 2 * F_TAB], bf16, kind="ExternalInput")
    dmat_d = nc.dram_tensor(
        "dmat", [P, P * 2 * G_SEL], bf16, kind="ExternalInput"
    )
    idx_d = [
        nc.dram_tensor(f"idx{g}", [P, n_idx[g]], i16, kind="ExternalInput")
        for g in range(nseg)
    ]
    n_cols = sum(_chunk_plan(C, g)[1] for g in range(nseg))
    dstloc_d = nc.dram_tensor("dstloc", [P, n_cols], bf16, kind="ExternalInput")
    out = nc.dram_tensor("out", [OWN, F_OUT], f32, kind="ExternalOutput")

    # ---- internal DRAM ----
    ag_in = nc.dram_tensor("ag_in", [OWN, F_TAB], bf16)
    # tabF[l]: the full node table (single 8-wide AllGather output,
    # node order == core order so the concat lands directly in layout)
    tabF = [
        nc.dram_tensor(f"tabF{l}", [N_NODES + 48, F_TAB], bf16)
        for l in range(3)
    ]

    bias_d = [b1, b2, b3]

    with tile.TileContext(nc) as tc:
        with (
            tc.tile_pool(name="const", bufs=1) as const_pool,
            tc.tile_pool(name="xt", bufs=1) as xt_pool,
            tc.tile_pool(name="hbuf", bufs=1) as h_pool,
            tc.tile_pool(name="hrelu", bufs=2) as hr_pool,
            tc.tile_pool(name="msg", bufs=MSG_BUFS) as msg_pool,
            tc.tile_pool(name="sel", bufs=3) as sel_pool,
            tc.tile_pool(name="small", bufs=3) as small_pool,
        ):
            # ---- preload constants ----
            dmat_sb = const_pool.tile([P, P, 2 * G_SEL], bf16, tag="dmat")
            nc.sync.dma_start(
                out=dmat_sb[:].rearrange("p a b -> p (a b)"), in_=dmat_d[:]
            )
            dis_sb = const_pool.tile([P, NBLK], f32, tag="dis")
            nc.sync.dma_start(out=dis_sb[:], in_=dis_own[:])
            W1_sb = const_pool.tile([F_IN, F_TAB], bf16, tag="w0")
            nc.sync.dma_start(out=W1_sb[:], in_=W1[:])
            bias_sb = []
            for l in range(3):
                b_t = const_pool.tile([P, F_TAB], f32, tag=f"b{l}")
                nc.sync.dma_start(out=b_t[:], in_=bias_d[l][:])
                bias_sb.append(b_t)
            Wr_sb = []
            for i, wd in enumerate((W2r, W3r)):
                w_t = const_pool.tile([P, 2, F_TAB], bf16, tag=f"wr{i}")
                nc.sync.dma_start(
                    out=w_t[:].rearrange("p a b -> p (a b)"), in_=wd[:]
                )
                Wr_sb.append(w_t)
            idx_sb = []
            for g in range(nseg):
                t_i = const_pool.tile([P, n_idx[g]], i16, tag=f"idx{g}")
                nc.sync.dma_start(out=t_i[:], in_=idx_d[g][:])
                idx_sb.append(t_i)
            dstloc_sb = const_pool.tile([P, n_cols], bf16, tag="dstloc")
            nc.sync.dma_start(out=dstloc_sb[:], in_=dstloc_d[:])
            xt_sb = xt_pool.tile([F_IN, NBLK * P], bf16, tag="xt")
            if OWN < NBLK * P:
                nc.vector.memset(xt_sb[:, OWN:], 0.0)
            nc.sync.dma_start(out=xt_sb[:, :OWN], in_=xT[:])

            def emit_halo_stage1(l):
                # single 8-wide AllGather: ag_in (this core's shard) ->
                # tabF[l] (the whole node table)
                if SIM_1CORE:
                    nc.sync.dma_start(out=tabF[l][:OWN, :], in_=ag_in[:])
                elif not SKIP_AG:
                    nc.gpsimd.collective_compute(
                        "AllGather",
                        mybir.AluOpType.bypass,
                        replica_groups=[[0, 1, 2, 3, 4, 5, 6, 7]],
                        ins=[ag_in[:].opt()],
                        outs=[tabF[l][:N_NODES, :].opt()],
                    )

            t_own = None  # [P, NBLK, F_TAB] f32; layer l's comes from l-1
            for l in range(L_MAX):
                if l == 0:
                    # ======== layer-0 transform from xT ============
                    t_own = h_pool.tile(
                        [P, NBLK, F_TAB], f32, tag="town", name="town_0"
                    )
                    agp = h_pool.tile(
                        [P, NBLK, F_TAB], bf16, tag="agp", name="agp_0"
                    )
                    with tc.tile_pool(name="tf", bufs=2, space="PSUM") as tf_pool:
                        for k in range(N_BANK):
                            nb = min(8, NBLK - 8 * k)
                            mmps = tf_pool.tile([P, 8, F_TAB], f32, tag="mm")
                            for j in range(nb):
                                b = 8 * k + j
                                nc.tensor.matmul(
                                    out=mmps[:, j, :],
                                    lhsT=xt_sb[:, b * P : (b + 1) * P],
                                    rhs=W1_sb[:],
                                    start=True,
                                    stop=True,
                                )
                            # batched scale by dis (per-block per-partition)
                            nc.vector.tensor_tensor(
                                out=t_own[:, 8 * k : 8 * k + nb, :],
                                in0=mmps[:, :nb, :],
                                in1=dis_sb[
                                    :, 8 * k : 8 * k + nb, None
                                ].to_broadcast([P, nb, F_TAB]),
                                op=mybir.AluOpType.mult,
                            )
                            nc.scalar.copy(
                                out=agp[:, 8 * k : 8 * k + nb, :],
                                in_=t_own[:, 8 * k : 8 * k + nb, :],
                            )
                    # packed shard -> ag_in (full blocks + tail)
                    nc.sync.dma_start(
                        out=ag_in[: (NBLK - 1) * P, :]
                        .rearrange("(b p) f -> p b f", p=P),
                        in_=agp[:, : NBLK - 1, :],
                    )
                    nc.sync.dma_start(
                        out=ag_in[(NBLK - 1) * P :, :]
                        .rearrange("(b p) f -> p b f", p=LAST_ROWS),
                        in_=agp[:LAST_ROWS, NBLK - 1 : NBLK, :],
                    )
                    emit_halo_stage1(0)
                # layers 1/2: t_own, ag_in and the stage-1 AG were all
                # produced inside layer l-1's aggregation (per-bank tails)
                # pair view: [*, 128] bf16 (256B rows)
                pairsF = tabF[l][:].rearrange("(a b) c -> a (b c)", b=2)
                # ======== aggregate into per-block PSUM ========
                with tc.tile_pool(name="banks", bufs=1, space="PSUM") as bk_pool:
                    banks = [
                        bk_pool.tile(
                            [P, 8, F_TAB], f32, tag=f"bank{k}", name=f"bank{k}_{l}"
                        )
                        for k in range(N_BANK)
                    ]
                    # per-layer epilogue + next-layer transform state
                    hsum = h_pool.tile(
                        [P, NBLK, F_TAB], f32, tag="hsum", name=f"hsum_{l}"
                    )
                    if l < L_MAX - 1:
                        # +1 pad block so DMA transposes batch in pairs
                        h_next = hr_pool.tile(
                            [P, NBLK + 1, F_TAB], bf16, tag="h",
                            name=f"h_{l}",
                        )
                        nc.vector.memset(h_next[:, NBLK:, :], 0.0)
                        t_next = h_pool.tile(
                            [P, NBLK, F_TAB], f32, tag="town",
                            name=f"town_{l + 1}",
                        )
                        agp_next = h_pool.tile(
                            [P, NBLK, F_TAB], bf16, tag="agp",
                            name=f"agp_{l + 1}",
                        )

                    def emit_tail(k, l=l):
                        """Epilogue for PSUM bank k; for l<2 also the
                        next-layer transform of its blocks + ag_in write.
                        Runs inside the seg-1 emission as each bank's last
                        accumulation chain closes, overlapping the tail
                        with the remaining gather-bound aggregation."""
                        nb = min(8, NBLK - 8 * k)
                        sl = slice(8 * k, 8 * k + nb)
                        nc.vector.tensor_tensor(
                            out=hsum[:, sl, :],
                            in0=banks[k][:, :nb, :],
                            in1=t_own[:, sl, :],
                            op=mybir.AluOpType.add,
                        )
                        nc.vector.tensor_tensor(
                            out=hsum[:, sl, :],
                            in0=hsum[:, sl, :],
                            in1=dis_sb[:, sl, None].to_broadcast(
                                [P, nb, F_TAB]
                            ),
                            op=mybir.AluOpType.mult,
                        )
                        nc.vector.tensor_tensor(
                            out=hsum[:, sl, :],
                            in0=hsum[:, sl, :],
                            in1=bias_sb[l][:, None, :].to_broadcast(
                                [P, nb, F_TAB]
                            ),
                            op=mybir.AluOpType.add,
                        )
                        if l == L_MAX - 1:
                            if k < N_BANK - 1:
                                nc.sync.dma_start(
                                    out=out[8 * k * P : (8 * k + nb) * P, :]
                                    .rearrange("(b p) f -> p b f", p=P),
                                    in_=hsum[:, sl, :F_OUT],
                                )
                            else:
                                nc.sync.dma_start(
                                    out=out[(NBLK - 1) * P :, :]
                                    .rearrange(
                                        "(b p) f -> p b f", p=LAST_ROWS
                                    ),
                                    in_=hsum[
                                        :LAST_ROWS, NBLK - 1 : NBLK, :F_OUT
                                    ],
                                )
                            return
                        nc.vector.tensor_scalar(
                            out=h_next[:, sl, :],
                            in0=hsum[:, sl, :],
                            scalar1=0.0,
                            scalar2=None,
                            op0=mybir.AluOpType.max,
                        )
                        # transform bank k for layer l+1: bf16 DMA
                        # transposes (2 blocks/xbar pass), matmuls reuse
                        # bank k's freshly-drained PSUM region
                        mmps = bk_pool.tile(
                            [P, 8, F_TAB], f32, tag=f"bank{k}",
                            name=f"mm{k}_{l}",
                        )
                        for q in range(0, nb, 2):
                            tr2 = small_pool.tile([P, P], bf16, tag="tr2")
                            nc.sync.dma_start_transpose(
                                out=tr2[:],
                                in_=h_next[:, 8 * k + q : 8 * k + q + 2, :],
                            )
                            for jj in range(2):
                                b2 = 8 * k + q + jj
                                if b2 >= NBLK:
                                    continue
                                nc.tensor.matmul(
                                    out=mmps[:, q + jj, :],
                                    lhsT=tr2[:, :],
                                    rhs=Wr_sb[l][:, jj, :],
                                    start=True,
                                    stop=True,
                                )
                        nc.vector.tensor_tensor(
                            out=t_next[:, sl, :],
                            in0=mmps[:, :nb, :],
                            in1=dis_sb[:, sl, None].to_broadcast(
                                [P, nb, F_TAB]
                            ),
                            op=mybir.AluOpType.mult,
                        )
                        nc.scalar.copy(
                            out=agp_next[:, sl, :],
                            in_=t_next[:, sl, :],
                        )
                        if k < N_BANK - 1:
                            nc.sync.dma_start(
                                out=ag_in[8 * k * P : (8 * k + nb) * P, :]
                                .rearrange("(b p) f -> p b f", p=P),
                                in_=agp_next[:, sl, :],
                            )
                        else:
                            nc.sync.dma_start(
                                out=ag_in[(NBLK - 1) * P :, :]
                                .rearrange("(b p) f -> p b f", p=LAST_ROWS),
                                in_=agp_next[
                                    :LAST_ROWS, NBLK - 1 : NBLK, :
                                ],
                            )
                            # NOTE: banks k<6 cover 8 full blocks; only the
                            # last bank holds the short tail block

                    agg_lo = None
                    col_base = 0
                    call_no = 0
                    for g in range(nseg):
                        in_view = pairsF[: N_NODES // 2 + 12, :]
                        plan, ncols_g = _chunk_plan(C, g)
                        n_ch = len(plan)
                        # NOTE: accumulation chains to one PSUM region must
                        # stay contiguous in PE order — interleaving open
                        # chains (deferred stop) corrupts results on HW —
                        # so each segment closes its chains and seg-0
                        # partials are evacuated to SBUF below.
                        tot_b = C[g].sum(axis=0)  # total chunks per block
                        ci = np.zeros(NBLK, dtype=np.int64)
                        for call0 in range(0, n_ch, G_CALL):
                            gcnt = min(G_CALL, n_ch - call0)
                            msg = msg_pool.tile(
                                [P, G_CALL, 2 * F_TAB], bf16, tag="msg"
                            )
                            geff = 1 if GATHER_OFF else gcnt
                            nc.gpsimd.dma_gather(
                                out_ap=msg[:, :geff, :],
                                in_ap=in_view,
                                idxs_ap=idx_sb[g][
                                    :, call0 * 8 : (call0 + geff) * 8
                                ],
                                num_idxs=geff * P,
                                num_idxs_reg=geff * P,
                                elem_size=2 * F_TAB,
                                single_packet=SINGLE_PACKET,
                                queue_num=call_no % N_QUEUES,
                            )
                            call_no += 1
                            for s0 in range(0, gcnt, G_SEL):
                                scnt = min(G_SEL, gcnt - s0)
                                first = plan[call0 + s0]
                                lastc = plan[call0 + s0 + scnt - 1]
                                col0 = first[2]
                                ncols = (
                                    lastc[2] + (2 if lastc[1] == 2 else 1)
                                    - col0
                                )
                                # sel layout [edge, dst, chunkcol]: all
                                # operands keep a real (stride-1, >=2)
                                # last dim, so the DVE runs this in the
                                # 2x_1p fast mode (a last-dim-broadcast
                                # operand would force 1x)
                                sel = sel_pool.tile(
                                    [P, P, 2 * G_SEL], bf16, tag="sel"
                                )
                                nc.vector.tensor_tensor(
                                    out=sel[:, :, :ncols],
                                    in0=dstloc_sb[
                                        :,
                                        None,
                                        col_base + col0 : col_base + col0
                                        + ncols,
                                    ].to_broadcast([P, P, ncols]),
                                    in1=dmat_sb[:, :, :ncols],
                                    op=mybir.AluOpType.is_equal,
                                )
                                for j in range(scnt):
                                    b, kind, colc = plan[call0 + s0 + j]
                                    lc = colc - col0
                                    last = bool(ci[b] == tot_b[b] - 1)
                                    if kind < 2:
                                        nc.tensor.matmul(
                                            out=banks[b // 8][:, b % 8, :],
                                            lhsT=sel[:, :, lc],
                                            rhs=msg[
                                                :,
                                                s0 + j,
                                                kind * F_TAB
                                                : (kind + 1) * F_TAB,
                                            ],
                                            start=bool(ci[b] == 0),
                                            stop=last,
                                        )
                                    else:
                                        nc.tensor.matmul(
                                            out=banks[b // 8][:, b % 8, :],
                                            lhsT=sel[:, :, lc],
                                            rhs=msg[:, s0 + j, 0:F_TAB],
                                            start=bool(ci[b] == 0),
                                            stop=False,
                                        )
                                        nc.tensor.matmul(
                                            out=banks[b // 8][:, b % 8, :],
                                            lhsT=sel[:, :, lc + 1],
                                            rhs=msg[
                                                :, s0 + j, F_TAB : 2 * F_TAB
                                            ],
                                            start=False,
                                            stop=last,
                                        )
                                    ci[b] += 1
                                    if (
                                        last
                                        and g == nseg - 1
                                        and b == min(8 * (b // 8) + 7,
                                                     NBLK - 1)
                                    ):
                                        # bank b//8's chains all closed (the
                                        # plan is block-ordered); no chain
                                        # is open right here, so the tail's
                                        # standalone matmuls can't split one
                                        emit_tail(b // 8)
                        col_base += ncols_g
                        if nseg == 2 and g == 0:
                            # evacuate first-phase partials to reuse banks
                            agg_lo = h_pool.tile(
                                [P, NBLK, F_TAB], f32, tag="agglo",
                                name=f"agglo_{l}",
                            )
                            for k in range(N_BANK):
                                nb = min(8, NBLK - 8 * k)
                                nc.scalar.copy(
                                    out=agg_lo[:, 8 * k : 8 * k + nb, :],
                                    in_=banks[k][:, :nb, :],
                                )
                    # ======== next layer's own-side exchange ========
                    if l < L_MAX - 1:
                        emit_halo_stage1(l + 1)
                        t_own = t_next

    nc.compile()
    return nc


def _stack_w(W):
    """[P, 2, F_TAB] bf16: slot 0 = [W; 0], slot 1 = [0; W] (see W2r)."""
    z = np.zeros((F_TAB, F_TAB), np.float32)
    a = np.concatenate([W, z], axis=0)  # [128, 64]
    b = np.concatenate([z, W], axis=0)
    return np.ascontiguousarray(
        np.stack([a, b], axis=1).reshape(P, 2 * F_TAB)
    ).astype(BF16)


def make_in_maps(x, W1, b1, W2, b2, W3, b3, dis, C, per_core):
    W3p = np.zeros((F_TAB, F_TAB), np.float32)
    W3p[:, :F_OUT] = np.asarray(W3, np.float32)
    b3p = np.zeros((F_TAB,), np.float32)
    b3p[:F_OUT] = np.asarray(b3, np.float32)
    # dmat[r, d, c] = d  (real stride-1 last dim for the DVE 2x sel build)
    dmat = np.broadcast_to(
        np.arange(P, dtype=np.float32)[None, :, None], (P, P, 2 * G_SEL)
    ).reshape(P, -1).astype(BF16).copy()

    in_maps = []
    for c in range(N_CORES):
        idx_segs, dstloc = per_core[c]
        d_own = dis[c * OWN : (c + 1) * OWN]
        pad = np.concatenate([d_own, np.ones(NBLK * P - OWN, np.float32)])
        m = {
            "xT": np.ascontiguousarray(
                x[c * OWN : (c + 1) * OWN].T
            ).astype(BF16),
            "W1": np.asarray(W1, np.float32).astype(BF16),
            "b1": np.broadcast_to(np.asarray(b1, np.float32), (P, F_TAB)).copy(),
            "b2": np.broadcast_to(np.asarray(b2, np.float32), (P, F_TAB)).copy(),
            "b3": np.broadcast_to(b3p, (P, F_TAB)).copy(),
            "dis_own": np.ascontiguousarray(pad.reshape(NBLK, P).T),
            "W2r": _stack_w(np.asarray(W2, np.float32)),
            "W3r": _stack_w(W3p),
            "dmat": dmat,
            "dstloc": dstloc,
        }
        for g in range(C.shape[0]):
            m[f"idx{g}"] = idx_segs[g]
        in_maps.append(m)
    return in_maps


_CACHE = {}


def kernel(x, edge_index, W1, b1, W2, b2, W3, b3):
    from concourse import bass_utils

    x = np.asarray(x, dtype=np.float32)
    edge_index = np.asarray(edge_index)
    key = hash(edge_index.tobytes())
    if key in _CACHE:
        nc, dis, segb, C, per_core, perm = _CACHE[key]
    else:
        perm = balance_permutation(edge_index)
        edge_perm = perm[np.asarray(edge_index, dtype=np.int64)]
        dis, segb, C, per_core = prep_graph(edge_perm)
        nc = build_program(C, segb, int(C.sum()))
        _CACHE[key] = (nc, dis, segb, C, per_core, perm)
    inv = np.argsort(perm)
    x = x[inv]  # x in new-id row order

    in_maps = make_in_maps(x, W1, b1, W2, b2, W3, b3, dis, C, per_core)

    res = bass_utils.run_bass_kernel_spmd(
        nc, in_maps, core_ids=list(range(N_CORES))
    )
    out = np.concatenate([res.results[c]["out"] for c in range(N_CORES)], axis=0)
    return out[perm]  # back to original node order

